# revision 26
# baseline (speedup 1.0000x reference)
"""Trainium2 Bass kernel for nn_NodeProcessor (GNN message passing), v2.

Strategy (8 NeuronCores, SPMD, no collectives):
  - Host sorts edges by destination node and shards NODES (6250/core);
    each core receives exactly the edges destined to its node shard, so no
    cross-core reduction is needed.
  - On device, segment-sum is computed per 128-node tile as a sequence of
    128-edge-chunk matmuls accumulating in PSUM (one-hot S matrices built
    on DVE by is_equal against iota constants; chunk 0 full width, later
    chunks a W=32 window at a host-baked offset).
  - Edge payload and the x MLP input are fp8 e3m4 (halves HBM traffic).
  - MLP: h1_T = relu(W1.T @ [x_T; agg_T] + b1) feature-major; h2 node-major
    via h1_T-stationary matmuls into a PSUM group buffer of LG=4 tiles.
  - LayerNorm per LG group directly on PSUM: one batched bn_stats + per-
    tile bn_aggr, rstd via ACT Sqrt + DVE reciprocal, apply as ACT
    Identity(in*rstd - mu*rstd) reading PSUM.  gamma-mult + residual-add
    (x + beta folded on host, bf16) on GpSimd per group; store per group.

v2 structural changes vs v1 (trace-driven):
  - All small constants packed into 2 bf16 + 1 f32 host tensors -> 3 DMA
    issues instead of 14 (each HWDGE dma_start costs ~610ns of sequencer).
  - DMA issue split across both HWDGE rings: Sync(SP) carries edge loads +
    output stores; Scalar(ACT) carries consts/xbf/xf.  xbf and xf are each
    ONE load instead of 13/7.
  - PE warm-up: ~28 dummy matmuls with zero deps issued first so the HAM
    clock-gate un-throttles before the real stream starts.
  - LN tail restructured: no PSUM->SBUF copy (stats+apply read PSUM),
    output stores per LG group (smaller tail), batched bn_stats.
"""

import os
import sys

import numpy as np

for _p in ("/opt/trn_rl_repo", "/root/.axon_site/_ro/trn_rl_repo"):
    if os.path.isdir(_p) and _p not in sys.path:
        sys.path.insert(0, _p)

import ml_dtypes

import concourse.bacc as bacc
import concourse.bass as bass
import concourse.tile as tile
from concourse import mybir
from concourse.bass_utils import run_bass_kernel_spmd
from concourse.tile import add_dep_helper

BF16 = ml_dtypes.bfloat16
FP8 = ml_dtypes.float8_e3m4

def _fuse_ldweights_in_bir(path):
    """Re-fuse Tile's split Ldweights+Matmult pairs into self-loading
    Matmults (bir.json level).  walrus's --enable-ldw-opt rejects any
    standalone InstLdweights; each Matmult already carries both operands,
    so the Ldweights rows are redundant once `ldweights` is set true.
    Waits on a dropped Ldweights merge into its Matmult."""
    import json

    with open(path) as fh:
        d = json.load(fh)
    nseq = [0]

    def nop_carrier(waits):
        nseq[0] += 1
        return {
            "debug": 0, "engine": "PE", "ins": [], "outs": [],
            "name": f"I-ldwfuse-{nseq[0]}", "opcode": "NoOp",
            "sync_info": {"on_update": [], "on_wait": waits},
            "text_hint": "ldwfuse_wait_carrier",
        }

    MAX_MM_WAITS = 1
    for fn in d["functions"]:
        for bb in fn["blocks"]:
            out = []
            pending = None
            for inst in bb["instructions"]:
                if inst.get("opcode") == "Ldweights":
                    si = inst.get("sync_info") or {}
                    assert not si.get("on_update"), "Ldweights with update"
                    if si.get("on_wait"):
                        pending = (pending or []) + list(si["on_wait"])
                    continue
                if inst.get("opcode") == "Matmult":
                    inst["ldweights"] = True
                    si = inst.setdefault(
                        "sync_info", {"on_wait": [], "on_update": []}
                    )
                    waits = list(si.get("on_wait", [])) + (pending or [])
                    pending = None
                    # dedup by (sem id, mode) keeping the max value
                    merged = {}
                    for w in waits:
                        k = (w["id"], w["wait_mode"])
                        if (k in merged
                                and merged[k]["wait_value"] >= w["wait_value"]):
                            continue
                        merged[k] = w
                    waits = list(merged.values())
                    # the fused LW command takes few waits: park the excess
                    # on a NoOp immediately before
                    if len(waits) > MAX_MM_WAITS:
                        out.append(nop_carrier(waits[MAX_MM_WAITS:]))
                        waits = waits[:MAX_MM_WAITS]
                    si["on_wait"] = waits
                elif pending is not None:
                    raise AssertionError(
                        f"Ldweights waits before {inst.get('opcode')}"
                    )
                out.append(inst)
            assert pending is None
            bb["instructions"] = out
    with open(path, "w") as fh:
        json.dump(d, fh)


if not os.environ.get("KERNEL_NO_LDW_OPT"):
    # Compile with the walrus LDW optimization (background weight-buffer
    # loads overlap the in-flight matmul).  walrus rejects standalone
    # InstLdweights under this flag, so the bir.json is rewritten to
    # re-fuse them into self-loading Matmults first.
    from concourse import bass_utils as _bu

    if not getattr(_bu, "_ldw_opt_patched", False):
        _bu._ldw_opt_patched = True
        _orig_run_command = _bu.run_command

        def _patched_run_command(argv, **kw):
            is_walrus = any("walrus_driver" in str(a) for a in argv[:1])
            if is_walrus:
                argv = [
                    "--enable-ldw-opt=true" if a == "--enable-ldw-opt=false"
                    else a for a in argv
                ]
                cwd = kw.get("cwd")
                bir = os.path.join(cwd or ".", "bir.json")
                if "-i" in argv and os.path.exists(bir):
                    _fuse_ldweights_in_bir(bir)
            return _orig_run_command(argv, **kw)

        _bu.run_command = _patched_run_command

N_NODES = 50000
N_EDGES = 600000
D = 128           # node/edge feature dim
H = 256           # hidden dim
NCORE = 8
NSHARD = N_NODES // NCORE      # 6250 real nodes per core
P = 128                        # partition / tile size
NT = 49                        # node tiles per core (49*128 = 6272 >= 6250)
G = 7                          # S0-build batch size (NT = G*G)
LG = 4                         # LN/store group size (tiles per PSUM bank)
NQ = -(-NT // LG)              # number of LG groups (13)
NPAD = NT * P                  # padded nodes per core
L = 32                         # edge chunks per DMA load
W = 32                         # scatter window width (max cross-core span 27)
SB = 16                        # windows per batched S-build op
LN_EPS = 1e-5
PAD_J = 200.0                  # j_rel sentinel for padded edge rows
N_WARMUP = 34                  # HAM warm-up matmuls (>3.4us to lift HAM)


def _prep_host(x, edge_index, edge_attr, W1, b1, W2, b2, ln_g, ln_b):
    """Sort/shard/pack all inputs."""
    j = np.asarray(edge_index[1], dtype=np.int64)
    perm = np.argsort(j, kind="stable")
    js = j[perm]

    edge_attr_q = np.asarray(edge_attr, dtype=FP8)
    x = np.asarray(x, dtype=np.float32)
    ln_b = np.asarray(ln_b, dtype=np.float32)

    bounds = np.searchsorted(js, np.arange(NCORE + 1) * NSHARD)

    core_info = []
    for c in range(NCORE):
        es, ee = bounds[c], bounds[c + 1]
        jl = js[es:ee] - c * NSHARD           # local node id, 0..6249
        rows = perm[es:ee]                    # rows into edge_attr
        cnt = np.bincount(jl // P, minlength=NT)  # edges per tile
        ch = -(-cnt // P)                     # ceil chunks per tile
        tile_perm = np.argsort(-ch, kind="stable")  # descending chunk count
        core_info.append((jl, rows, cnt, ch, tile_perm))

    sorted_ch = np.stack([ci[3][ci[4]] for ci in core_info])  # [NCORE, NT]
    schedule = np.maximum(sorted_ch.max(axis=0), 1).astype(np.int64)
    nchunk = int(schedule.sum())
    nload = -(-nchunk // L)
    nc_tot = nload * L

    chunk_base = np.zeros(NT + 1, dtype=np.int64)
    np.cumsum(schedule, out=chunk_base[1:])

    # Tile-relative j_rel per chunk slot per core; chunk 0 of a tile is
    # full-width, later chunks use a common W-wide window.
    minj = np.full((NCORE, nc_tot), 1 << 30, dtype=np.int64)
    maxj = np.full((NCORE, nc_tot), -1, dtype=np.int64)
    per_core_fill = []
    for c in range(NCORE):
        jl, rows, cnt, ch, tile_perm = core_info[c]
        tile_start = np.zeros(NT + 1, dtype=np.int64)
        np.cumsum(cnt, out=tile_start[1:])
        ridx = np.zeros(nc_tot * P, dtype=np.int64)
        jrel_t = np.full(nc_tot * P, -1, dtype=np.int64)  # tile-relative
        for s in range(NT):
            T = int(tile_perm[s])
            n = int(cnt[T])
            dst = chunk_base[s] * P
            ridx[dst : dst + n] = rows[tile_start[T] : tile_start[T] + n]
            jrel_t[dst : dst + n] = jl[tile_start[T] : tile_start[T] + n] - T * P
        jr2 = jrel_t.reshape(nc_tot, P)
        valid = jr2 >= 0
        anyv = valid.any(axis=1)
        mn = np.where(anyv, np.where(valid, jr2, 1 << 30).min(axis=1), 1 << 30)
        mx = np.where(anyv, np.where(valid, jr2, -1).max(axis=1), -1)
        minj[c] = mn
        maxj[c] = mx
        per_core_fill.append((ridx, jrel_t))

    woff = np.clip(minj.min(axis=0), 0, P - W)
    woff[chunk_base[:-1]] = 0  # chunk 0 full width
    fw = np.zeros(nc_tot, dtype=bool)
    fw[chunk_base[:-1]] = True
    width = np.where(fw, P, W)
    assert (maxj.max(axis=0) < woff + width).all(), "chunk span exceeds window"

    b2_zero = bool(np.all(np.asarray(b2) == 0))

    in_maps = []
    for c in range(NCORE):
        jl, rows, cnt, ch, tile_perm = core_info[c]
        ridx, jrel_t = per_core_fill[c]
        jr2 = jrel_t.reshape(nc_tot, P).astype(np.float32) - woff[:, None]
        jr2[jrel_t.reshape(nc_tot, P) < 0] = PAD_J

        ea_all = edge_attr_q[ridx]            # [nc_tot*P, D] fp8
        ea_pack = (
            ea_all.reshape(nload, L, P, D)
            .transpose(0, 2, 1, 3)
            .reshape(nload, P, L * D)
            .copy()
        )
        jr_pack = np.ascontiguousarray(jr2.T.astype(BF16))  # [P, nc_tot]
        # chunk-0 columns (tile-relative j_rel) gathered into slot order
        jr0_pack = np.ascontiguousarray(jr2[chunk_base[:-1]].T.astype(BF16))
        iotaw = np.tile(
            np.repeat(np.arange(W, dtype=np.float32), SB), (P, 1)
        ).astype(BF16)
        iotag = np.tile(
            np.repeat(np.arange(P, dtype=np.float32), G), (P, 1)
        ).astype(BF16)
        # const pack A (S-build deps): jr | jr0 | iotaw | iotag
        cbfA = np.concatenate([jr_pack, jr0_pack, iotaw, iotag], axis=1)

        # const pack B (MLP deps): gb | W1 quads | W2 halves
        gb = np.tile(np.asarray(ln_g, np.float32), (P, 1)).astype(BF16)
        W1b = np.asarray(W1, BF16)
        W2b = np.asarray(W2, BF16)
        cbfB = np.concatenate(
            [gb,
             W1b[0:P, 0:P], W1b[0:P, P:2*P],
             W1b[P:2*P, 0:P], W1b[P:2*P, P:2*P],
             W2b[0:P, :], W2b[P:2*P, :]],
            axis=1,
        )
        cf32 = np.ascontiguousarray(
            np.asarray(b1, np.float32).reshape(2, P).T
        )  # [P, 2]: col0=b1[:128], col1=b1[128:]

        # x shard: fp8 feature-major (MLP input) and bf16 node-major
        # residual (+ beta folded), both in tile_perm slot order.
        xs = np.zeros((NPAD, D), dtype=np.float32)
        xs[:NSHARD] = x[c * NSHARD : (c + 1) * NSHARD]
        xt = xs.reshape(NT, P, D).transpose(0, 2, 1)[tile_perm]  # [NT, f, n]
        xtq = np.zeros((NQ * LG, D, P), dtype=np.float32)
        xtq[:NT] = xt
        # one tensor [D, NQ*LG*P], quad-major cols
        xbf_pack = np.ascontiguousarray(
            xtq.astype(FP8).transpose(1, 0, 2).reshape(D, NQ * LG * P)
        )
        xfn = (xs + ln_b[None, :]).reshape(NT, P, D)[tile_perm]  # [NT, n, f]
        xf_pack = np.ascontiguousarray(
            xfn.astype(BF16).transpose(1, 0, 2).reshape(P, NT * D)
        )

        m = {
            "ea": ea_pack,
            "cbfA": cbfA,
            "cbfB": cbfB,
            "cf32": cf32,
            "xbf": xbf_pack,
            "xf": xf_pack,
        }
        if not b2_zero:
            m["b2g"] = np.tile(np.asarray(b2, BF16).reshape(1, D), (1, LG))
        in_maps.append(m)

    meta = (schedule, woff, nload, nc_tot, b2_zero)
    return in_maps, meta, [ci[4] for ci in core_info]


def _build_program(meta):
    schedule, woff, nload, nc_tot, b2_zero = meta
    f32 = mybir.dt.float32
    bf16 = mybir.dt.bfloat16
    fp8 = mybir.dt.float8e3
    AF = mybir.ActivationFunctionType
    OP = mybir.AluOpType

    nc = bacc.Bacc("TRN2", target_bir_lowering=False, debug=False,
                   num_devices=NCORE)

    NCA = nc_tot + NT + W * SB + P * G
    NCB = D + 6 * P
    ea_d = nc.dram_tensor("ea", [nload, P, L * D], fp8, kind="ExternalInput").ap()
    cbfA_d = nc.dram_tensor("cbfA", [P, NCA], bf16, kind="ExternalInput").ap()
    cbfB_d = nc.dram_tensor("cbfB", [P, NCB], bf16, kind="ExternalInput").ap()
    cf32_d = nc.dram_tensor("cf32", [P, 2], f32, kind="ExternalInput").ap()
    xbf_d = nc.dram_tensor("xbf", [D, NQ * LG * P], fp8, kind="ExternalInput").ap()
    xf_d = nc.dram_tensor("xf", [P, NT * D], bf16, kind="ExternalInput").ap()
    if not b2_zero:
        b2g_d = nc.dram_tensor("b2g", [1, LG * D], bf16, kind="ExternalInput").ap()
    out_d = nc.dram_tensor("outN", [NQ, P, LG * D], bf16, kind="ExternalOutput").ap()

    with tile.TileContext(nc) as tc:
        with (
            tc.tile_pool(name="consts", bufs=1) as consts,
            tc.tile_pool(name="edges", bufs=6) as epool,
            tc.tile_pool(name="xg", bufs=2) as xpool,
            tc.tile_pool(name="yg", bufs=3) as ypool,
            tc.tile_pool(name="s0", bufs=3) as s0pool,
            tc.tile_pool(name="sm", bufs=18) as spool,
            tc.tile_pool(name="work", bufs=3) as wpool,
            tc.tile_pool(name="ln", bufs=3) as lnpool,
            tc.tile_pool(name="ps", bufs=1, space="PSUM") as pspool,
            tc.tile_pool(name="ps2", bufs=3, space="PSUM") as ps2pool,
            tc.tile_pool(name="psagg", bufs=3, space="PSUM") as psagg,
        ):
            def _pe_touch(producer_inst):
                """Hoist a stationary-operand wait off Ldweights: a PE NOP
                artificially depending on the producer carries the sem wait,
                so later Ldweights are wait-free (needed for ldw-opt)."""
                nop = nc.tensor.nop(nofuse=True, hint="ldw_wait_hoist")
                add_dep_helper(nop.ins, producer_inst.ins,
                               reason="ldw-opt: wait on PE nop, not Ldweights")

            # ---- PE warm-up: zero-dep matmul stream to lift the HAM gate
            # (shares the psagg ring; its bank is recycled by scatter tile 2)
            wz = consts.tile([P, P], bf16, tag="wz")
            _pe_touch(nc.vector.memset(wz[:], 0.0))
            wups = psagg.tile([P, P], f32, tag="agg")
            for i in range(N_WARMUP):
                nc.tensor.matmul(wups[:], lhsT=wz[:], rhs=wz[:],
                                 start=(i == 0), stop=(i == N_WARMUP - 1))

            # ---- constants (scalar=ACT HWDGE ring) ----
            # xbf/xf loads are EMITTED later (at quad 0) so their HBM
            # traffic doesn't contend with the first edge loads.
            cA = consts.tile([P, NCA], bf16, tag="cA")
            nc.scalar.dma_start(out=cA[:], in_=cbfA_d[:])
            cB = consts.tile([P, NCB], bf16, tag="cB")
            _pe_touch(nc.scalar.dma_start(out=cB[:], in_=cbfB_d[:]))
            cf = consts.tile([P, 2], f32, tag="cf")
            nc.scalar.dma_start(out=cf[:], in_=cf32_d[:])
            xbf_sb = consts.tile([D, NQ * LG * P], fp8, tag="xbf")
            xf_sb = consts.tile([P, NT * D], bf16, tag="xf")

            o = 0
            jr_sb = cA[:, o:o + nc_tot]; o += nc_tot
            jr0_sb = cA[:, o:o + NT]; o += NT
            iotaw_sb = cA[:, o:o + W * SB]; o += W * SB
            iotag_sb = cA[:, o:o + P * G]
            o = 0
            gb_sb = cB[:, o:o + D]; o += D
            w1xa = cB[:, o:o + P]; o += P
            w1xb = cB[:, o:o + P]; o += P
            w1ga = cB[:, o:o + P]; o += P
            w1gb = cB[:, o:o + P]; o += P
            w2a = cB[:, o:o + P]; o += P
            w2b = cB[:, o:o + P]
            b1a = cf[:, 0:1]
            b1b = cf[:, 1:2]

            eps_sb = consts.tile([P, 1], f32, tag="eps")
            nc.vector.memset(eps_sb[:], LN_EPS)
            if not b2_zero:
                ones_row = consts.tile([1, P], bf16, tag="ones_row")
                nc.vector.memset(ones_row[:], 1.0)
                b2g_sb = consts.tile([1, LG * D], bf16, tag="b2g")
                nc.scalar.dma_start(out=b2g_sb[:], in_=b2g_d[:])

            def mid_bcast(a, shape):
                """AP broadcasting a [P, k] slice to [P, shape[1], k]."""
                return bass.AP(
                    tensor=a.tensor, offset=a.offset,
                    ap=[a.ap[0], [0, shape[1]], a.ap[1]],
                )

            load_tiles = {}

            def ensure_load(ld):
                if ld < 0 or ld >= nload or ld in load_tiles:
                    return
                et = epool.tile([P, L * D], fp8, tag="ea", name=f"ea{ld}")
                # alternate HWDGE rings (first 3 on sync to get the body
                # started; scalar ring is free after the 3 const issues)
                eng = nc.sync if (ld < 3 or ld % 2 == 0) else nc.scalar
                _pe_touch(eng.dma_start(out=et[:], in_=ea_d[ld]))
                load_tiles[ld] = et

            def edge_slice(c):
                ld, sl = divmod(c, L)
                ensure_load(ld)
                ensure_load(ld + 1)
                ensure_load(ld + 2)
                return load_tiles[ld][:, sl * D : (sl + 1) * D]

            chunk_base = np.zeros(NT + 1, dtype=np.int64)
            np.cumsum(schedule, out=chunk_base[1:])

            # batched full-width S for the chunk-0s of G tiles,
            # layout [e, n, t] (t innermost -> 2x mode)
            s0_tiles = {}

            def s0_group(gi):
                if gi not in s0_tiles:
                    S0g = s0pool.tile([P, P * G], bf16, tag="S0g")
                    jr0s = jr0_sb[:, gi * G : (gi + 1) * G]
                    nc.vector.tensor_tensor(
                        out=S0g[:].rearrange("p (n t) -> p n t", t=G),
                        in0=mid_bcast(jr0s, [P, P, G]),
                        in1=iotag_sb.rearrange("p (n t) -> p n t", t=G),
                        op=OP.is_equal,
                    )
                    s0_tiles[gi] = S0g
                return s0_tiles[gi]

            def s0_rhs(gi, ti):
                S0g = s0_group(gi)
                a = S0g[:]
                return bass.AP(tensor=a.tensor, offset=a.offset + ti,
                               ap=[a.ap[0], [G, P]])

            aggT_pairs = {}
            s_of = {}

            def sbuild_tile(t):
                """Selection matrices for tile t, layout [e, w, q]."""
                c0 = int(chunk_base[t])
                ncch = int(schedule[t])
                s0_group(t // G)
                sbs = []
                for q0 in range(1, ncch, SB):
                    qn = min(SB, ncch - q0)
                    Sb = spool.tile([P, W * SB], bf16, tag="Sb",
                                    name=f"Sb{t}_{q0}")
                    jrs = jr_sb[:, c0 + q0 : c0 + q0 + qn]
                    nc.vector.tensor_tensor(
                        out=Sb[:, : W * qn].rearrange("p (w q) -> p w q", q=qn),
                        in0=mid_bcast(jrs, [P, W, qn]),
                        in1=bass.AP(tensor=iotaw_sb.tensor,
                                    offset=iotaw_sb.offset,
                                    ap=[iotaw_sb.ap[0], [SB, W], [1, qn]]),
                        op=OP.is_equal,
                    )
                    sbs.append((Sb, qn))
                s_of[t] = sbs

            def win_rhs(Sb, qn, i):
                a = Sb[:]
                return bass.AP(tensor=a.tensor, offset=a.offset + i,
                               ap=[a.ap[0], [qn, W]])

            def scatter_tile(t):
                gi, ti = divmod(t, G)
                c0 = int(chunk_base[t])
                ncch = int(schedule[t])
                agg_ps = psagg.tile([P, P], f32, tag="agg")
                nc.tensor.matmul(
                    agg_ps[:], lhsT=edge_slice(c0), rhs=s0_rhs(gi, ti),
                    start=True, stop=(ncch == 1),
                )
                sbs = s_of.pop(t)
                for bi, q0 in enumerate(range(1, ncch, SB)):
                    Sb, qn = sbs[bi]
                    for i in range(qn):
                        c = c0 + q0 + i
                        w = int(woff[c])
                        nc.tensor.matmul(
                            agg_ps[:, w : w + W],
                            lhsT=edge_slice(c),
                            rhs=win_rhs(Sb, qn, i),
                            start=False,
                            stop=(c == c0 + ncch - 1),
                            skip_group_check=True,
                        )
                # copy to SBUF so the PSUM bank frees early; quads of tiles
                # share one SBUF tile so h1 can batch over all four.
                # GpSimd cannot read PSUM; split copies between ACT and DVE.
                p, half = divmod(t, LG)
                if half == 0:
                    aggT_pairs[p] = wpool.tile([P, LG * P], bf16, tag="aggT",
                                               name=f"aggT{p}")
                dst = aggT_pairs[p][:, half * P : (half + 1) * P]
                if t % 2 == 0:
                    nc.scalar.activation(out=dst, in_=agg_ps[:],
                                         func=AF.Copy, bias=0.0, scale=1.0)
                else:
                    nc.vector.tensor_copy(out=dst, in_=agg_ps[:])

            def mlp_h1_quad(p):
                """h1 for tiles 4p..4p+3 batched over the node axis."""
                t0 = LG * p
                nt = min(LG, NT - t0)
                aggT = aggT_pairs.pop(p)
                NN = nt * P
                xT = xbf_sb[:, p * LG * P : p * LG * P + NN]

                h1a_ps = pspool.tile([P, LG * P], f32, tag="h1a")
                nc.tensor.matmul(h1a_ps[:, 0:NN], lhsT=w1xa, rhs=xT,
                                 start=True, stop=False)
                nc.tensor.matmul(h1a_ps[:, 0:NN], lhsT=w1ga,
                                 rhs=aggT[:, 0:NN], start=False, stop=True)
                h1a = wpool.tile([P, LG * P], bf16, tag="h1a_sb")
                _pe_touch(nc.scalar.activation(
                    out=h1a[:, 0:NN], in_=h1a_ps[:, 0:NN],
                    func=AF.Relu, bias=b1a, scale=1.0))

                h1b_ps = pspool.tile([P, LG * P], f32, tag="h1b")
                nc.tensor.matmul(h1b_ps[:, 0:NN], lhsT=w1xb, rhs=xT,
                                 start=True, stop=False)
                nc.tensor.matmul(h1b_ps[:, 0:NN], lhsT=w1gb,
                                 rhs=aggT[:, 0:NN], start=False, stop=True)
                h1b = wpool.tile([P, LG * P], bf16, tag="h1b_sb")
                _pe_touch(nc.scalar.activation(
                    out=h1b[:, 0:NN], in_=h1b_ps[:, 0:NN],
                    func=AF.Relu, bias=b1b, scale=1.0))
                return h1a, h1b

            # ---- h2 into a 4-tile PSUM group, LN tail per group ----
            ln_state = {}

            def h2_tile(t, h1a, h1b, half):
                lg, li = divmod(t, LG)
                if li == 0:
                    ln_state[lg] = ps2pool.tile([P, LG * P], f32, tag="h2g",
                                                name=f"h2g{lg}")
                h2g = ln_state[lg]
                sl = slice(li * P, (li + 1) * P)
                # start=True clears the has_written bits of the whole PSUM
                # BANK, so only the group's first matmul may set it; later
                # slices rely on the bank-wide clear (first write with
                # start=False overwrites where has_written=0)
                nc.tensor.matmul(h2g[:, sl],
                                 lhsT=h1a[:, half * P : (half + 1) * P],
                                 rhs=w2a, start=(li == 0), stop=False,
                                 skip_group_check=(li != 0))
                last = (li == LG - 1) or (t == NT - 1)
                nc.tensor.matmul(h2g[:, sl],
                                 lhsT=h1b[:, half * P : (half + 1) * P],
                                 rhs=w2b, start=False,
                                 stop=(b2_zero and last),
                                 skip_group_check=True)

            def ln_group(lg):
                """b2 + LayerNorm + gamma + residual + store, tiles
                [4*lg, 4*lg+nt)."""
                t0 = lg * LG
                nt = min(LG, NT - t0)
                h2g = ln_state[lg]
                NN = nt * P
                # rank-1 b2 add over the whole group, closes all accum
                # groups.  Skipped when b2 == 0.
                if not b2_zero:
                    nc.tensor.matmul(h2g[:, 0:NN], lhsT=ones_row[:],
                                     rhs=b2g_sb[:, 0:NN], start=False,
                                     stop=True, skip_group_check=True)
                # LN stats directly on PSUM: one batched bn_stats, per-tile
                # bn_aggr
                stats = lnpool.tile([P, LG * 6], f32, tag="stats")
                mv = lnpool.tile([P, LG * 2], f32, tag="mv")
                for i in range(nt):
                    nc.vector.bn_stats(out=stats[:, 6 * i : 6 * i + 6],
                                       in_=h2g[:, i * P : (i + 1) * P])
                    nc.vector.bn_aggr(out=mv[:, 2 * i : 2 * i + 2],
                                      in_=stats[:, 6 * i : 6 * i + 6])
                mva = mv[:]
                var_sl = bass.AP(tensor=mva.tensor, offset=mva.offset + 1,
                                 ap=[mva.ap[0], [2, nt]])
                mean_sl = bass.AP(tensor=mva.tensor, offset=mva.offset,
                                  ap=[mva.ap[0], [2, nt]])
                rstd = lnpool.tile([P, LG], f32, tag="rstd")
                nc.scalar.activation(out=rstd[:, 0:nt], in_=var_sl,
                                     func=AF.Sqrt, bias=eps_sb[:], scale=1.0)
                nc.vector.reciprocal(out=rstd[:, 0:nt], in_=rstd[:, 0:nt])
                # nmr = -mu * rstd  (bias for the ACT Identity apply)
                nmr = lnpool.tile([P, LG], f32, tag="nmr")
                nc.vector.tensor_tensor(out=nmr[:, 0:nt], in0=mean_sl,
                                        in1=rstd[:, 0:nt], op=OP.mult)
                nc.vector.tensor_scalar(out=nmr[:, 0:nt], in0=nmr[:, 0:nt],
                                        scalar1=-1.0, scalar2=None,
                                        op0=OP.mult)
                # u = v*rstd - mu*rstd per tile, reading PSUM.  Alternate
                # ACT(Identity) / DVE(tensor_scalar) by group parity so
                # consecutive groups' LN chains overlap across engines.
                u_g = lnpool.tile([P, LG * D], bf16, tag="u_g")
                for i in range(nt):
                    if lg % 2 == 0:
                        nc.scalar.activation(
                            out=u_g[:, i * D : (i + 1) * D],
                            in_=h2g[:, i * P : (i + 1) * P],
                            func=AF.Identity,
                            bias=nmr[:, i : i + 1],
                            scale=rstd[:, i : i + 1],
                        )
                    else:
                        nc.vector.tensor_scalar(
                            out=u_g[:, i * D : (i + 1) * D],
                            in0=h2g[:, i * P : (i + 1) * P],
                            scalar1=rstd[:, i : i + 1],
                            scalar2=nmr[:, i : i + 1],
                            op0=OP.mult, op1=OP.add,
                        )
                del ln_state[lg]
                # gamma mult + residual add: GpSimd for the steady state,
                # DVE for the last groups (short tail); store ring by parity
                t2 = ypool.tile([P, LG * D], bf16, tag="t2")
                gba = gb_sb
                veng = nc.vector if lg >= NQ - 2 else nc.gpsimd
                veng.tensor_tensor(
                    out=t2[:, 0:NN].rearrange("p (t f) -> p t f", f=D),
                    in0=u_g[:, 0:NN].rearrange("p (t f) -> p t f", f=D),
                    in1=bass.AP(tensor=gba.tensor, offset=gba.offset,
                                ap=[gba.ap[0], [0, nt], [1, D]]),
                    op=OP.mult,
                )
                y_g = ypool.tile([P, LG * D], bf16, tag="yg")
                veng.tensor_tensor(out=y_g[:, 0:NN], in0=t2[:, 0:NN],
                                   in1=xf_sb[:, t0 * D : t0 * D + NN],
                                   op=OP.add)
                seng = nc.sync if lg % 2 == 0 else nc.scalar
                seng.dma_start(out=out_d[lg][:, 0:NN], in_=y_g[:, 0:NN])

            # software pipeline: S-builds run SA tiles ahead of the scatter
            # matmuls, which run MA tiles ahead of the MLP.  MA=4 keeps
            # scatter matmuls flowing into the last quads (aggT for quad p
            # completes during quad p-1).
            SA, MA = 16, 4
            for t in range(min(SA, NT)):
                sbuild_tile(t)
            for t in range(min(MA, NT)):
                scatter_tile(t)
            for p in range(NQ):
                t0 = LG * p
                quad = [t for t in range(t0, t0 + LG) if t < NT]
                for t in quad:
                    if t + SA < NT:
                        sbuild_tile(t + SA)
                if p == 0:
                    # xbf (quad 0 h1 input) and xf (ln_group residual) are
                    # needed from here on; issuing now keeps their HBM
                    # traffic off the critical first edge loads
                    nc.scalar.dma_start(out=xbf_sb[:], in_=xbf_d[:])
                    nc.scalar.dma_start(out=xf_sb[:], in_=xf_d[:])
                h1a, h1b = mlp_h1_quad(p)
                for t in quad:
                    if t + MA < NT:
                        scatter_tile(t + MA)
                for t in quad:
                    h2_tile(t, h1a, h1b, t - t0)
                ln_group(p)

    nc.finalize()
    return nc


def _strip_ldw_waits(nc):
    """Move sem waits off InstLdweights onto the nearest preceding PE
    instruction (usually the ldw_wait_hoist NOP emitted next to it).
    walrus's --enable-ldw-opt rejects Ldweights carrying semaphores; the
    move is conservative (the wait fires earlier in the same engine
    stream) so ordering is preserved."""
    f = nc.m.functions[0]
    moved = 0
    for bb in f.blocks:
        prev_pe = None
        for inst in bb.instructions:
            if isinstance(inst, mybir.InstLdweights):
                si = inst.sync_info
                waits = list(si.on_wait) if si is not None else []
                if waits:
                    assert prev_pe is not None, "Ldweights first on PE"
                    import bass_rust
                    psi = prev_pe.sync_info
                    if psi is None:
                        prev_pe.sync_info = bass_rust.SyncInfo(
                            on_wait=waits, on_update=[])
                    else:
                        merged = {}
                        for w in list(psi.on_wait) + waits:
                            k = (w.id, str(w.wait_mode))
                            if (k in merged
                                    and merged[k].wait_value >= w.wait_value):
                                continue
                            merged[k] = w
                        psi.on_wait = list(merged.values())
                    si.on_wait = []
                    moved += 1
            elif inst.engine == mybir.EngineType.PE:
                prev_pe = inst


LAST_RESULT = None


def kernel(x, edge_index, edge_attr, W1, b1, W2, b2, ln_g, ln_b):
    global LAST_RESULT
    in_maps, meta, tile_perms = _prep_host(
        x, edge_index, edge_attr, W1, b1, W2, b2, ln_g, ln_b
    )
    nc = _build_program(meta)
    trace = bool(os.environ.get("KERNEL_TRACE"))
    res = run_bass_kernel_spmd(
        nc, in_maps, core_ids=list(range(NCORE)), trace=trace
    )
    LAST_RESULT = res

    out = np.empty((N_NODES, D), dtype=np.float32)
    for c in range(NCORE):
        yN = np.asarray(res.results[c]["outN"], dtype=np.float32)
        y_slots = (
            yN.reshape(NQ, P, LG, D).transpose(0, 2, 1, 3)
            .reshape(NQ * LG, P, D)[:NT]
        )
        y_tiles = np.empty_like(y_slots)
        y_tiles[tile_perms[c]] = y_slots
        y = y_tiles.reshape(NPAD, D)[:NSHARD]
        out[c * NSHARD : (c + 1) * NSHARD] = y
    return out


# revision 27
# speedup vs baseline: 1.0476x; 1.0476x over previous
"""Trainium2 Bass kernel for nn_NodeProcessor (GNN message passing), v2.

Strategy (8 NeuronCores, SPMD, no collectives):
  - Host sorts edges by destination node and shards NODES (6250/core);
    each core receives exactly the edges destined to its node shard, so no
    cross-core reduction is needed.
  - On device, segment-sum is computed per 128-node tile as a sequence of
    128-edge-chunk matmuls accumulating in PSUM (one-hot S matrices built
    on DVE by is_equal against iota constants; chunk 0 full width, later
    chunks a W=32 window at a host-baked offset).
  - Edge payload and the x MLP input are fp8 e3m4 (halves HBM traffic).
  - MLP: h1_T = relu(W1.T @ [x_T; agg_T] + b1) feature-major; h2 node-major
    via h1_T-stationary matmuls into a PSUM group buffer of LG=4 tiles.
  - LayerNorm per LG group directly on PSUM: one batched bn_stats + per-
    tile bn_aggr, rstd via ACT Sqrt + DVE reciprocal, apply as ACT
    Identity(in*rstd - mu*rstd) reading PSUM.  gamma-mult + residual-add
    (x + beta folded on host, bf16) on GpSimd per group; store per group.

v2 structural changes vs v1 (trace-driven):
  - All small constants packed into 2 bf16 + 1 f32 host tensors -> 3 DMA
    issues instead of 14 (each HWDGE dma_start costs ~610ns of sequencer).
  - DMA issue split across both HWDGE rings: Sync(SP) carries edge loads +
    output stores; Scalar(ACT) carries consts/xbf/xf.  xbf and xf are each
    ONE load instead of 13/7.
  - PE warm-up: ~28 dummy matmuls with zero deps issued first so the HAM
    clock-gate un-throttles before the real stream starts.
  - LN tail restructured: no PSUM->SBUF copy (stats+apply read PSUM),
    output stores per LG group (smaller tail), batched bn_stats.
"""

import os
import sys

import numpy as np

for _p in ("/opt/trn_rl_repo", "/root/.axon_site/_ro/trn_rl_repo"):
    if os.path.isdir(_p) and _p not in sys.path:
        sys.path.insert(0, _p)

import ml_dtypes

import concourse.bacc as bacc
import concourse.bass as bass
import concourse.tile as tile
from concourse import mybir
from concourse.bass_utils import run_bass_kernel_spmd

BF16 = ml_dtypes.bfloat16
FP8 = ml_dtypes.float8_e3m4

if os.environ.get("KERNEL_LDW_OPT"):
    from concourse import bass_utils as _bu

    _orig_run_command = _bu.run_command

    def _patched_run_command(argv, **kw):
        argv = [
            "--enable-ldw-opt=true" if a == "--enable-ldw-opt=false" else a
            for a in argv
        ]
        return _orig_run_command(argv, **kw)

    _bu.run_command = _patched_run_command

N_NODES = 50000
N_EDGES = 600000
D = 128           # node/edge feature dim
H = 256           # hidden dim
NCORE = 8
NSHARD = N_NODES // NCORE      # 6250 real nodes per core
P = 128                        # partition / tile size
NT = 49                        # node tiles per core (49*128 = 6272 >= 6250)
G = 7                          # S0-build batch size (NT = G*G)
LG = 4                         # LN/store group size (tiles per PSUM bank)
NQ = -(-NT // LG)              # number of LG groups (13)
NPAD = NT * P                  # padded nodes per core
L = 32                         # edge chunks per DMA load
W = 32                         # scatter window width (max cross-core span 27)
SB = 16                        # windows per batched S-build op
LN_EPS = 1e-5
PAD_J = 200.0                  # j_rel sentinel for padded edge rows
N_WARMUP = 28                  # HAM warm-up matmuls


def _prep_host(x, edge_index, edge_attr, W1, b1, W2, b2, ln_g, ln_b):
    """Sort/shard/pack all inputs."""
    j = np.asarray(edge_index[1], dtype=np.int64)
    perm = np.argsort(j, kind="stable")
    js = j[perm]

    edge_attr_q = np.asarray(edge_attr, dtype=FP8)
    x = np.asarray(x, dtype=np.float32)
    ln_b = np.asarray(ln_b, dtype=np.float32)

    bounds = np.searchsorted(js, np.arange(NCORE + 1) * NSHARD)

    core_info = []
    for c in range(NCORE):
        es, ee = bounds[c], bounds[c + 1]
        jl = js[es:ee] - c * NSHARD           # local node id, 0..6249
        rows = perm[es:ee]                    # rows into edge_attr
        cnt = np.bincount(jl // P, minlength=NT)  # edges per tile
        ch = -(-cnt // P)                     # ceil chunks per tile
        tile_perm = np.argsort(-ch, kind="stable")  # descending chunk count
        core_info.append((jl, rows, cnt, ch, tile_perm))

    sorted_ch = np.stack([ci[3][ci[4]] for ci in core_info])  # [NCORE, NT]
    schedule = np.maximum(sorted_ch.max(axis=0), 1).astype(np.int64)
    nchunk = int(schedule.sum())
    nload = -(-nchunk // L)
    nc_tot = nload * L

    chunk_base = np.zeros(NT + 1, dtype=np.int64)
    np.cumsum(schedule, out=chunk_base[1:])

    # Tile-relative j_rel per chunk slot per core; chunk 0 of a tile is
    # full-width, later chunks use a common W-wide window.
    minj = np.full((NCORE, nc_tot), 1 << 30, dtype=np.int64)
    maxj = np.full((NCORE, nc_tot), -1, dtype=np.int64)
    per_core_fill = []
    for c in range(NCORE):
        jl, rows, cnt, ch, tile_perm = core_info[c]
        tile_start = np.zeros(NT + 1, dtype=np.int64)
        np.cumsum(cnt, out=tile_start[1:])
        ridx = np.zeros(nc_tot * P, dtype=np.int64)
        jrel_t = np.full(nc_tot * P, -1, dtype=np.int64)  # tile-relative
        for s in range(NT):
            T = int(tile_perm[s])
            n = int(cnt[T])
            dst = chunk_base[s] * P
            ridx[dst : dst + n] = rows[tile_start[T] : tile_start[T] + n]
            jrel_t[dst : dst + n] = jl[tile_start[T] : tile_start[T] + n] - T * P
        jr2 = jrel_t.reshape(nc_tot, P)
        valid = jr2 >= 0
        anyv = valid.any(axis=1)
        mn = np.where(anyv, np.where(valid, jr2, 1 << 30).min(axis=1), 1 << 30)
        mx = np.where(anyv, np.where(valid, jr2, -1).max(axis=1), -1)
        minj[c] = mn
        maxj[c] = mx
        per_core_fill.append((ridx, jrel_t))

    woff = np.clip(minj.min(axis=0), 0, P - W)
    woff[chunk_base[:-1]] = 0  # chunk 0 full width
    fw = np.zeros(nc_tot, dtype=bool)
    fw[chunk_base[:-1]] = True
    width = np.where(fw, P, W)
    assert (maxj.max(axis=0) < woff + width).all(), "chunk span exceeds window"

    b2_zero = bool(np.all(np.asarray(b2) == 0))

    in_maps = []
    for c in range(NCORE):
        jl, rows, cnt, ch, tile_perm = core_info[c]
        ridx, jrel_t = per_core_fill[c]
        jr2 = jrel_t.reshape(nc_tot, P).astype(np.float32) - woff[:, None]
        jr2[jrel_t.reshape(nc_tot, P) < 0] = PAD_J

        ea_all = edge_attr_q[ridx]            # [nc_tot*P, D] fp8
        ea_pack = (
            ea_all.reshape(nload, L, P, D)
            .transpose(0, 2, 1, 3)
            .reshape(nload, P, L * D)
            .copy()
        )
        jr_pack = np.ascontiguousarray(jr2.T.astype(BF16))  # [P, nc_tot]
        # chunk-0 columns (tile-relative j_rel) gathered into slot order
        jr0_pack = np.ascontiguousarray(jr2[chunk_base[:-1]].T.astype(BF16))
        iotaw = np.tile(
            np.repeat(np.arange(W, dtype=np.float32), SB), (P, 1)
        ).astype(BF16)
        iotag = np.tile(
            np.repeat(np.arange(P, dtype=np.float32), G), (P, 1)
        ).astype(BF16)
        # const pack A (S-build deps): jr | jr0 | iotaw | iotag
        cbfA = np.concatenate([jr_pack, jr0_pack, iotaw, iotag], axis=1)

        # const pack B (MLP deps): gb | W1 quads | W2 halves
        gb = np.tile(np.asarray(ln_g, np.float32), (P, 1)).astype(BF16)
        W1b = np.asarray(W1, BF16)
        W2b = np.asarray(W2, BF16)
        cbfB = np.concatenate(
            [gb,
             W1b[0:P, 0:P], W1b[0:P, P:2*P],
             W1b[P:2*P, 0:P], W1b[P:2*P, P:2*P],
             W2b[0:P, :], W2b[P:2*P, :]],
            axis=1,
        )
        cf32 = np.ascontiguousarray(
            np.asarray(b1, np.float32).reshape(2, P).T
        )  # [P, 2]: col0=b1[:128], col1=b1[128:]

        # x shard: fp8 feature-major (MLP input) and bf16 node-major
        # residual (+ beta folded), both in tile_perm slot order.
        xs = np.zeros((NPAD, D), dtype=np.float32)
        xs[:NSHARD] = x[c * NSHARD : (c + 1) * NSHARD]
        xt = xs.reshape(NT, P, D).transpose(0, 2, 1)[tile_perm]  # [NT, f, n]
        xtq = np.zeros((NQ * LG, D, P), dtype=np.float32)
        xtq[:NT] = xt
        # one tensor [D, NQ*LG*P], quad-major cols
        xbf_pack = np.ascontiguousarray(
            xtq.astype(FP8).transpose(1, 0, 2).reshape(D, NQ * LG * P)
        )
        xfn = (xs + ln_b[None, :]).reshape(NT, P, D)[tile_perm]  # [NT, n, f]
        xf_pack = np.ascontiguousarray(
            xfn.astype(BF16).transpose(1, 0, 2).reshape(P, NT * D)
        )

        m = {
            "ea": ea_pack,
            "cbfA": cbfA,
            "cbfB": cbfB,
            "cf32": cf32,
            "xbf": xbf_pack,
            "xf": xf_pack,
        }
        if not b2_zero:
            m["b2g"] = np.tile(np.asarray(b2, BF16).reshape(1, D), (1, LG))
        in_maps.append(m)

    meta = (schedule, woff, nload, nc_tot, b2_zero)
    return in_maps, meta, [ci[4] for ci in core_info]


def _build_program(meta):
    schedule, woff, nload, nc_tot, b2_zero = meta
    f32 = mybir.dt.float32
    bf16 = mybir.dt.bfloat16
    fp8 = mybir.dt.float8e3
    AF = mybir.ActivationFunctionType
    OP = mybir.AluOpType

    nc = bacc.Bacc("TRN2", target_bir_lowering=False, debug=False,
                   num_devices=NCORE)

    NCA = nc_tot + NT + W * SB + P * G
    NCB = D + 6 * P
    ea_d = nc.dram_tensor("ea", [nload, P, L * D], fp8, kind="ExternalInput").ap()
    cbfA_d = nc.dram_tensor("cbfA", [P, NCA], bf16, kind="ExternalInput").ap()
    cbfB_d = nc.dram_tensor("cbfB", [P, NCB], bf16, kind="ExternalInput").ap()
    cf32_d = nc.dram_tensor("cf32", [P, 2], f32, kind="ExternalInput").ap()
    xbf_d = nc.dram_tensor("xbf", [D, NQ * LG * P], fp8, kind="ExternalInput").ap()
    xf_d = nc.dram_tensor("xf", [P, NT * D], bf16, kind="ExternalInput").ap()
    if not b2_zero:
        b2g_d = nc.dram_tensor("b2g", [1, LG * D], bf16, kind="ExternalInput").ap()
    out_d = nc.dram_tensor("outN", [NQ, P, LG * D], bf16, kind="ExternalOutput").ap()

    with tile.TileContext(nc) as tc:
        with (
            tc.tile_pool(name="consts", bufs=1) as consts,
            tc.tile_pool(name="edges", bufs=6) as epool,
            tc.tile_pool(name="xg", bufs=2) as xpool,
            tc.tile_pool(name="yg", bufs=3) as ypool,
            tc.tile_pool(name="s0", bufs=3) as s0pool,
            tc.tile_pool(name="sm", bufs=18) as spool,
            tc.tile_pool(name="work", bufs=3) as wpool,
            tc.tile_pool(name="ln", bufs=3) as lnpool,
            tc.tile_pool(name="ps", bufs=1, space="PSUM") as pspool,
            tc.tile_pool(name="ps2", bufs=3, space="PSUM") as ps2pool,
            tc.tile_pool(name="psagg", bufs=3, space="PSUM") as psagg,
        ):
            # ---- PE warm-up: zero-dep matmul stream to lift the HAM gate
            # (shares the psagg ring; its bank is recycled by scatter tile 2)
            wz = consts.tile([P, P], bf16, tag="wz")
            nc.vector.memset(wz[:], 0.0)
            wups = psagg.tile([P, P], f32, tag="agg")
            for i in range(N_WARMUP):
                nc.tensor.matmul(wups[:], lhsT=wz[:], rhs=wz[:],
                                 start=(i == 0), stop=(i == N_WARMUP - 1))

            # ---- constants (scalar=ACT HWDGE ring) ----
            cA = consts.tile([P, NCA], bf16, tag="cA")
            nc.scalar.dma_start(out=cA[:], in_=cbfA_d[:])
            cB = consts.tile([P, NCB], bf16, tag="cB")
            nc.scalar.dma_start(out=cB[:], in_=cbfB_d[:])
            cf = consts.tile([P, 2], f32, tag="cf")
            nc.scalar.dma_start(out=cf[:], in_=cf32_d[:])
            xbf_sb = consts.tile([D, NQ * LG * P], fp8, tag="xbf")
            nc.scalar.dma_start(out=xbf_sb[:], in_=xbf_d[:])
            xf_sb = consts.tile([P, NT * D], bf16, tag="xf")
            nc.scalar.dma_start(out=xf_sb[:], in_=xf_d[:])

            o = 0
            jr_sb = cA[:, o:o + nc_tot]; o += nc_tot
            jr0_sb = cA[:, o:o + NT]; o += NT
            iotaw_sb = cA[:, o:o + W * SB]; o += W * SB
            iotag_sb = cA[:, o:o + P * G]
            o = 0
            gb_sb = cB[:, o:o + D]; o += D
            w1xa = cB[:, o:o + P]; o += P
            w1xb = cB[:, o:o + P]; o += P
            w1ga = cB[:, o:o + P]; o += P
            w1gb = cB[:, o:o + P]; o += P
            w2a = cB[:, o:o + P]; o += P
            w2b = cB[:, o:o + P]
            b1a = cf[:, 0:1]
            b1b = cf[:, 1:2]

            eps_sb = consts.tile([P, 1], f32, tag="eps")
            nc.vector.memset(eps_sb[:], LN_EPS)
            if not b2_zero:
                ones_row = consts.tile([1, P], bf16, tag="ones_row")
                nc.vector.memset(ones_row[:], 1.0)
                b2g_sb = consts.tile([1, LG * D], bf16, tag="b2g")
                nc.scalar.dma_start(out=b2g_sb[:], in_=b2g_d[:])

            def mid_bcast(a, shape):
                """AP broadcasting a [P, k] slice to [P, shape[1], k]."""
                return bass.AP(
                    tensor=a.tensor, offset=a.offset,
                    ap=[a.ap[0], [0, shape[1]], a.ap[1]],
                )

            load_tiles = {}

            def ensure_load(ld):
                if ld < 0 or ld >= nload or ld in load_tiles:
                    return
                et = epool.tile([P, L * D], fp8, tag="ea", name=f"ea{ld}")
                nc.sync.dma_start(out=et[:], in_=ea_d[ld])
                load_tiles[ld] = et

            def edge_slice(c):
                ld, sl = divmod(c, L)
                ensure_load(ld)
                ensure_load(ld + 1)
                ensure_load(ld + 2)
                return load_tiles[ld][:, sl * D : (sl + 1) * D]

            chunk_base = np.zeros(NT + 1, dtype=np.int64)
            np.cumsum(schedule, out=chunk_base[1:])

            # batched full-width S for the chunk-0s of G tiles,
            # layout [e, n, t] (t innermost -> 2x mode)
            s0_tiles = {}

            def s0_group(gi):
                if gi not in s0_tiles:
                    S0g = s0pool.tile([P, P * G], bf16, tag="S0g")
                    jr0s = jr0_sb[:, gi * G : (gi + 1) * G]
                    nc.vector.tensor_tensor(
                        out=S0g[:].rearrange("p (n t) -> p n t", t=G),
                        in0=mid_bcast(jr0s, [P, P, G]),
                        in1=iotag_sb.rearrange("p (n t) -> p n t", t=G),
                        op=OP.is_equal,
                    )
                    s0_tiles[gi] = S0g
                return s0_tiles[gi]

            def s0_rhs(gi, ti):
                S0g = s0_group(gi)
                a = S0g[:]
                return bass.AP(tensor=a.tensor, offset=a.offset + ti,
                               ap=[a.ap[0], [G, P]])

            aggT_pairs = {}
            s_of = {}

            def sbuild_tile(t):
                """Selection matrices for tile t, layout [e, w, q]."""
                c0 = int(chunk_base[t])
                ncch = int(schedule[t])
                s0_group(t // G)
                sbs = []
                for q0 in range(1, ncch, SB):
                    qn = min(SB, ncch - q0)
                    Sb = spool.tile([P, W * SB], bf16, tag="Sb",
                                    name=f"Sb{t}_{q0}")
                    jrs = jr_sb[:, c0 + q0 : c0 + q0 + qn]
                    nc.vector.tensor_tensor(
                        out=Sb[:, : W * qn].rearrange("p (w q) -> p w q", q=qn),
                        in0=mid_bcast(jrs, [P, W, qn]),
                        in1=bass.AP(tensor=iotaw_sb.tensor,
                                    offset=iotaw_sb.offset,
                                    ap=[iotaw_sb.ap[0], [SB, W], [1, qn]]),
                        op=OP.is_equal,
                    )
                    sbs.append((Sb, qn))
                s_of[t] = sbs

            def win_rhs(Sb, qn, i):
                a = Sb[:]
                return bass.AP(tensor=a.tensor, offset=a.offset + i,
                               ap=[a.ap[0], [qn, W]])

            def scatter_tile(t):
                gi, ti = divmod(t, G)
                c0 = int(chunk_base[t])
                ncch = int(schedule[t])
                agg_ps = psagg.tile([P, P], f32, tag="agg")
                nc.tensor.matmul(
                    agg_ps[:], lhsT=edge_slice(c0), rhs=s0_rhs(gi, ti),
                    start=True, stop=(ncch == 1),
                )
                sbs = s_of.pop(t)
                for bi, q0 in enumerate(range(1, ncch, SB)):
                    Sb, qn = sbs[bi]
                    for i in range(qn):
                        c = c0 + q0 + i
                        w = int(woff[c])
                        nc.tensor.matmul(
                            agg_ps[:, w : w + W],
                            lhsT=edge_slice(c),
                            rhs=win_rhs(Sb, qn, i),
                            start=False,
                            stop=(c == c0 + ncch - 1),
                            skip_group_check=True,
                        )
                # copy to SBUF so the PSUM bank frees early; quads of tiles
                # share one SBUF tile so h1 can batch over all four.
                # GpSimd cannot read PSUM; split copies between ACT and DVE.
                p, half = divmod(t, LG)
                if half == 0:
                    aggT_pairs[p] = wpool.tile([P, LG * P], bf16, tag="aggT",
                                               name=f"aggT{p}")
                dst = aggT_pairs[p][:, half * P : (half + 1) * P]
                if t % 2 == 0:
                    nc.scalar.activation(out=dst, in_=agg_ps[:],
                                         func=AF.Copy, bias=0.0, scale=1.0)
                else:
                    nc.vector.tensor_copy(out=dst, in_=agg_ps[:])

            def mlp_h1_quad(p):
                """h1 for tiles 4p..4p+3 batched over the node axis."""
                t0 = LG * p
                nt = min(LG, NT - t0)
                aggT = aggT_pairs.pop(p)
                NN = nt * P
                xT = xbf_sb[:, p * LG * P : p * LG * P + NN]

                h1a_ps = pspool.tile([P, LG * P], f32, tag="h1a")
                nc.tensor.matmul(h1a_ps[:, 0:NN], lhsT=w1xa, rhs=xT,
                                 start=True, stop=False)
                nc.tensor.matmul(h1a_ps[:, 0:NN], lhsT=w1ga,
                                 rhs=aggT[:, 0:NN], start=False, stop=True)
                h1a = wpool.tile([P, LG * P], bf16, tag="h1a_sb")
                nc.scalar.activation(out=h1a[:, 0:NN], in_=h1a_ps[:, 0:NN],
                                     func=AF.Relu, bias=b1a, scale=1.0)

                h1b_ps = pspool.tile([P, LG * P], f32, tag="h1b")
                nc.tensor.matmul(h1b_ps[:, 0:NN], lhsT=w1xb, rhs=xT,
                                 start=True, stop=False)
                nc.tensor.matmul(h1b_ps[:, 0:NN], lhsT=w1gb,
                                 rhs=aggT[:, 0:NN], start=False, stop=True)
                h1b = wpool.tile([P, LG * P], bf16, tag="h1b_sb")
                nc.scalar.activation(out=h1b[:, 0:NN], in_=h1b_ps[:, 0:NN],
                                     func=AF.Relu, bias=b1b, scale=1.0)
                return h1a, h1b

            # ---- h2 into a 4-tile PSUM group, LN tail per group ----
            ln_state = {}

            def h2_tile(t, h1a, h1b, half):
                lg, li = divmod(t, LG)
                if li == 0:
                    ln_state[lg] = ps2pool.tile([P, LG * P], f32, tag="h2g",
                                                name=f"h2g{lg}")
                h2g = ln_state[lg]
                sl = slice(li * P, (li + 1) * P)
                # start=True clears the has_written bits of the whole PSUM
                # BANK, so only the group's first matmul may set it; later
                # slices rely on the bank-wide clear (first write with
                # start=False overwrites where has_written=0)
                nc.tensor.matmul(h2g[:, sl],
                                 lhsT=h1a[:, half * P : (half + 1) * P],
                                 rhs=w2a, start=(li == 0), stop=False,
                                 skip_group_check=(li != 0))
                last = (li == LG - 1) or (t == NT - 1)
                nc.tensor.matmul(h2g[:, sl],
                                 lhsT=h1b[:, half * P : (half + 1) * P],
                                 rhs=w2b, start=False,
                                 stop=(b2_zero and last),
                                 skip_group_check=True)

            def ln_group(lg):
                """b2 + LayerNorm + gamma + residual + store, tiles
                [4*lg, 4*lg+nt)."""
                t0 = lg * LG
                nt = min(LG, NT - t0)
                h2g = ln_state[lg]
                NN = nt * P
                # rank-1 b2 add over the whole group, closes all accum
                # groups.  Skipped when b2 == 0.
                if not b2_zero:
                    nc.tensor.matmul(h2g[:, 0:NN], lhsT=ones_row[:],
                                     rhs=b2g_sb[:, 0:NN], start=False,
                                     stop=True, skip_group_check=True)
                # LN stats directly on PSUM: one batched bn_stats, per-tile
                # bn_aggr
                stats = lnpool.tile([P, LG * 6], f32, tag="stats")
                mv = lnpool.tile([P, LG * 2], f32, tag="mv")
                for i in range(nt):
                    nc.vector.bn_stats(out=stats[:, 6 * i : 6 * i + 6],
                                       in_=h2g[:, i * P : (i + 1) * P])
                    nc.vector.bn_aggr(out=mv[:, 2 * i : 2 * i + 2],
                                      in_=stats[:, 6 * i : 6 * i + 6])
                mva = mv[:]
                var_sl = bass.AP(tensor=mva.tensor, offset=mva.offset + 1,
                                 ap=[mva.ap[0], [2, nt]])
                mean_sl = bass.AP(tensor=mva.tensor, offset=mva.offset,
                                  ap=[mva.ap[0], [2, nt]])
                rstd = lnpool.tile([P, LG], f32, tag="rstd")
                nc.scalar.activation(out=rstd[:, 0:nt], in_=var_sl,
                                     func=AF.Sqrt, bias=eps_sb[:], scale=1.0)
                nc.vector.reciprocal(out=rstd[:, 0:nt], in_=rstd[:, 0:nt])
                # nmr = -mu * rstd  (bias for the ACT Identity apply)
                nmr = lnpool.tile([P, LG], f32, tag="nmr")
                nc.vector.tensor_tensor(out=nmr[:, 0:nt], in0=mean_sl,
                                        in1=rstd[:, 0:nt], op=OP.mult)
                nc.vector.tensor_scalar(out=nmr[:, 0:nt], in0=nmr[:, 0:nt],
                                        scalar1=-1.0, scalar2=None,
                                        op0=OP.mult)
                # u = v*rstd - mu*rstd per tile on ACT, reading PSUM
                u_g = lnpool.tile([P, LG * D], bf16, tag="u_g")
                for i in range(nt):
                    nc.scalar.activation(
                        out=u_g[:, i * D : (i + 1) * D],
                        in_=h2g[:, i * P : (i + 1) * P],
                        func=AF.Identity,
                        bias=nmr[:, i : i + 1],
                        scale=rstd[:, i : i + 1],
                    )
                del ln_state[lg]
                # gamma mult + residual add on GpSimd, store on sync ring
                t2 = ypool.tile([P, LG * D], bf16, tag="t2")
                gba = gb_sb
                nc.gpsimd.tensor_tensor(
                    out=t2[:, 0:NN].rearrange("p (t f) -> p t f", f=D),
                    in0=u_g[:, 0:NN].rearrange("p (t f) -> p t f", f=D),
                    in1=bass.AP(tensor=gba.tensor, offset=gba.offset,
                                ap=[gba.ap[0], [0, nt], [1, D]]),
                    op=OP.mult,
                )
                y_g = ypool.tile([P, LG * D], bf16, tag="yg")
                nc.gpsimd.tensor_tensor(out=y_g[:, 0:NN], in0=t2[:, 0:NN],
                                        in1=xf_sb[:, t0 * D : t0 * D + NN],
                                        op=OP.add)
                nc.sync.dma_start(out=out_d[lg][:, 0:NN], in_=y_g[:, 0:NN])

            # software pipeline: S-builds run SA tiles ahead of the scatter
            # matmuls, which run MA tiles ahead of the MLP
            SA, MA = 16, 6
            for t in range(min(SA, NT)):
                sbuild_tile(t)
            for t in range(min(MA, NT)):
                scatter_tile(t)
            for p in range(NQ):
                t0 = LG * p
                quad = [t for t in range(t0, t0 + LG) if t < NT]
                for t in quad:
                    if t + SA < NT:
                        sbuild_tile(t + SA)
                h1a, h1b = mlp_h1_quad(p)
                for t in quad:
                    if t + MA < NT:
                        scatter_tile(t + MA)
                for t in quad:
                    h2_tile(t, h1a, h1b, t - t0)
                ln_group(p)

    nc.finalize()
    return nc


LAST_RESULT = None


def kernel(x, edge_index, edge_attr, W1, b1, W2, b2, ln_g, ln_b):
    global LAST_RESULT
    in_maps, meta, tile_perms = _prep_host(
        x, edge_index, edge_attr, W1, b1, W2, b2, ln_g, ln_b
    )
    nc = _build_program(meta)
    trace = bool(os.environ.get("KERNEL_TRACE"))
    res = run_bass_kernel_spmd(
        nc, in_maps, core_ids=list(range(NCORE)), trace=trace
    )
    LAST_RESULT = res

    out = np.empty((N_NODES, D), dtype=np.float32)
    for c in range(NCORE):
        yN = np.asarray(res.results[c]["outN"], dtype=np.float32)
        y_slots = (
            yN.reshape(NQ, P, LG, D).transpose(0, 2, 1, 3)
            .reshape(NQ * LG, P, D)[:NT]
        )
        y_tiles = np.empty_like(y_slots)
        y_tiles[tile_perms[c]] = y_slots
        y = y_tiles.reshape(NPAD, D)[:NSHARD]
        out[c * NSHARD : (c + 1) * NSHARD] = y
    return out


# revision 28
# speedup vs baseline: 1.0674x; 1.0189x over previous
"""Trainium2 Bass kernel for nn_NodeProcessor (GNN message passing), v2.

Strategy (8 NeuronCores, SPMD, no collectives):
  - Host sorts edges by destination node and shards NODES (6250/core);
    each core receives exactly the edges destined to its node shard, so no
    cross-core reduction is needed.
  - On device, segment-sum is computed per 128-node tile as a sequence of
    128-edge-chunk matmuls accumulating in PSUM (one-hot S matrices built
    on DVE by is_equal against iota constants; chunk 0 full width, later
    chunks a W=32 window at a host-baked offset).
  - Edge payload and the x MLP input are fp8 e3m4 (halves HBM traffic).
  - MLP: h1_T = relu(W1.T @ [x_T; agg_T] + b1) feature-major; h2 node-major
    via h1_T-stationary matmuls into a PSUM group buffer of LG=4 tiles.
  - LayerNorm per LG group directly on PSUM: one batched bn_stats + per-
    tile bn_aggr, rstd via ACT Sqrt + DVE reciprocal, apply as ACT
    Identity(in*rstd - mu*rstd) reading PSUM.  gamma-mult + residual-add
    (x + beta folded on host, bf16) on GpSimd per group; store per group.

v2 structural changes vs v1 (trace-driven):
  - All small constants packed into 2 bf16 + 1 f32 host tensors -> 3 DMA
    issues instead of 14 (each HWDGE dma_start costs ~610ns of sequencer).
  - DMA issue split across both HWDGE rings: Sync(SP) carries edge loads +
    output stores; Scalar(ACT) carries consts/xbf/xf.  xbf and xf are each
    ONE load instead of 13/7.
  - PE warm-up: ~28 dummy matmuls with zero deps issued first so the HAM
    clock-gate un-throttles before the real stream starts.
  - LN tail restructured: no PSUM->SBUF copy (stats+apply read PSUM),
    output stores per LG group (smaller tail), batched bn_stats.
"""

import os
import sys

import numpy as np

for _p in ("/opt/trn_rl_repo", "/root/.axon_site/_ro/trn_rl_repo"):
    if os.path.isdir(_p) and _p not in sys.path:
        sys.path.insert(0, _p)

import ml_dtypes

import concourse.bacc as bacc
import concourse.bass as bass
import concourse.tile as tile
from concourse import mybir
from concourse.bass_utils import run_bass_kernel_spmd

BF16 = ml_dtypes.bfloat16
FP8 = ml_dtypes.float8_e3m4

if os.environ.get("KERNEL_LDW_OPT"):
    from concourse import bass_utils as _bu

    _orig_run_command = _bu.run_command

    def _patched_run_command(argv, **kw):
        argv = [
            "--enable-ldw-opt=true" if a == "--enable-ldw-opt=false" else a
            for a in argv
        ]
        return _orig_run_command(argv, **kw)

    _bu.run_command = _patched_run_command

N_NODES = 50000
N_EDGES = 600000
D = 128           # node/edge feature dim
H = 256           # hidden dim
NCORE = 8
NSHARD = N_NODES // NCORE      # 6250 real nodes per core
P = 128                        # partition / tile size
NT = 49                        # node tiles per core (49*128 = 6272 >= 6250)
G = 7                          # S0-build batch size (NT = G*G)
LG = 4                         # LN/store group size (tiles per PSUM bank)
NQ = -(-NT // LG)              # number of LG groups (13)
NPAD = NT * P                  # padded nodes per core
L = 32                         # edge chunks per DMA load
W = 32                         # scatter window width (max cross-core span 27)
SB = 16                        # windows per batched S-build op
LN_EPS = 1e-5
PAD_J = 200.0                  # j_rel sentinel for padded edge rows
N_WARMUP = 34                  # HAM warm-up matmuls (>=3.4us busy window)


def _prep_host(x, edge_index, edge_attr, W1, b1, W2, b2, ln_g, ln_b):
    """Sort/shard/pack all inputs."""
    j = np.asarray(edge_index[1], dtype=np.int64)
    perm = np.argsort(j, kind="stable")
    js = j[perm]

    edge_attr_q = np.asarray(edge_attr, dtype=FP8)
    x = np.asarray(x, dtype=np.float32)
    ln_b = np.asarray(ln_b, dtype=np.float32)

    bounds = np.searchsorted(js, np.arange(NCORE + 1) * NSHARD)

    core_info = []
    for c in range(NCORE):
        es, ee = bounds[c], bounds[c + 1]
        jl = js[es:ee] - c * NSHARD           # local node id, 0..6249
        rows = perm[es:ee]                    # rows into edge_attr
        cnt = np.bincount(jl // P, minlength=NT)  # edges per tile
        ch = -(-cnt // P)                     # ceil chunks per tile
        tile_perm = np.argsort(-ch, kind="stable")  # descending chunk count
        core_info.append((jl, rows, cnt, ch, tile_perm))

    sorted_ch = np.stack([ci[3][ci[4]] for ci in core_info])  # [NCORE, NT]
    schedule = np.maximum(sorted_ch.max(axis=0), 1).astype(np.int64)
    nchunk = int(schedule.sum())
    nload = -(-nchunk // L)
    nc_tot = nload * L

    chunk_base = np.zeros(NT + 1, dtype=np.int64)
    np.cumsum(schedule, out=chunk_base[1:])

    # Tile-relative j_rel per chunk slot per core; chunk 0 of a tile is
    # full-width, later chunks use a common W-wide window.
    minj = np.full((NCORE, nc_tot), 1 << 30, dtype=np.int64)
    maxj = np.full((NCORE, nc_tot), -1, dtype=np.int64)
    per_core_fill = []
    for c in range(NCORE):
        jl, rows, cnt, ch, tile_perm = core_info[c]
        tile_start = np.zeros(NT + 1, dtype=np.int64)
        np.cumsum(cnt, out=tile_start[1:])
        ridx = np.zeros(nc_tot * P, dtype=np.int64)
        jrel_t = np.full(nc_tot * P, -1, dtype=np.int64)  # tile-relative
        for s in range(NT):
            T = int(tile_perm[s])
            n = int(cnt[T])
            dst = chunk_base[s] * P
            ridx[dst : dst + n] = rows[tile_start[T] : tile_start[T] + n]
            jrel_t[dst : dst + n] = jl[tile_start[T] : tile_start[T] + n] - T * P
        jr2 = jrel_t.reshape(nc_tot, P)
        valid = jr2 >= 0
        anyv = valid.any(axis=1)
        mn = np.where(anyv, np.where(valid, jr2, 1 << 30).min(axis=1), 1 << 30)
        mx = np.where(anyv, np.where(valid, jr2, -1).max(axis=1), -1)
        minj[c] = mn
        maxj[c] = mx
        per_core_fill.append((ridx, jrel_t))

    woff = np.clip(minj.min(axis=0), 0, P - W)
    woff[chunk_base[:-1]] = 0  # chunk 0 full width
    fw = np.zeros(nc_tot, dtype=bool)
    fw[chunk_base[:-1]] = True
    width = np.where(fw, P, W)
    assert (maxj.max(axis=0) < woff + width).all(), "chunk span exceeds window"

    b2_zero = bool(np.all(np.asarray(b2) == 0))

    in_maps = []
    for c in range(NCORE):
        jl, rows, cnt, ch, tile_perm = core_info[c]
        ridx, jrel_t = per_core_fill[c]
        jr2 = jrel_t.reshape(nc_tot, P).astype(np.float32) - woff[:, None]
        jr2[jrel_t.reshape(nc_tot, P) < 0] = PAD_J

        ea_all = edge_attr_q[ridx]            # [nc_tot*P, D] fp8
        ea_pack = (
            ea_all.reshape(nload, L, P, D)
            .transpose(0, 2, 1, 3)
            .reshape(nload, P, L * D)
            .copy()
        )
        jr_pack = np.ascontiguousarray(jr2.T.astype(BF16))  # [P, nc_tot]
        # chunk-0 columns (tile-relative j_rel) gathered into slot order
        jr0_pack = np.ascontiguousarray(jr2[chunk_base[:-1]].T.astype(BF16))
        iotaw = np.tile(
            np.repeat(np.arange(W, dtype=np.float32), SB), (P, 1)
        ).astype(BF16)
        iotag = np.tile(
            np.repeat(np.arange(P, dtype=np.float32), G), (P, 1)
        ).astype(BF16)
        # const pack A (S-build deps): jr | jr0 | iotaw | iotag
        cbfA = np.concatenate([jr_pack, jr0_pack, iotaw, iotag], axis=1)

        # const pack B (MLP deps): gb | W1 quads | W2 halves
        gb = np.tile(np.asarray(ln_g, np.float32), (P, 1)).astype(BF16)
        W1b = np.asarray(W1, BF16)
        W2b = np.asarray(W2, BF16)
        cbfB = np.concatenate(
            [gb,
             W1b[0:P, 0:P], W1b[0:P, P:2*P],
             W1b[P:2*P, 0:P], W1b[P:2*P, P:2*P],
             W2b[0:P, :], W2b[P:2*P, :]],
            axis=1,
        )
        cf32 = np.ascontiguousarray(
            np.asarray(b1, np.float32).reshape(2, P).T
        )  # [P, 2]: col0=b1[:128], col1=b1[128:]

        # x shard: fp8 feature-major (MLP input) and bf16 node-major
        # residual (+ beta folded), both in tile_perm slot order.
        xs = np.zeros((NPAD, D), dtype=np.float32)
        xs[:NSHARD] = x[c * NSHARD : (c + 1) * NSHARD]
        xt = xs.reshape(NT, P, D).transpose(0, 2, 1)[tile_perm]  # [NT, f, n]
        xtq = np.zeros((NQ * LG, D, P), dtype=np.float32)
        xtq[:NT] = xt
        # one tensor [D, NQ*LG*P], quad-major cols
        xbf_pack = np.ascontiguousarray(
            xtq.astype(FP8).transpose(1, 0, 2).reshape(D, NQ * LG * P)
        )
        xfn = (xs + ln_b[None, :]).reshape(NT, P, D)[tile_perm]  # [NT, n, f]
        xf_pack = np.ascontiguousarray(
            xfn.astype(BF16).transpose(1, 0, 2).reshape(P, NT * D)
        )

        m = {
            "ea": ea_pack,
            "cbfA": cbfA,
            "cbfB": cbfB,
            "cf32": cf32,
            "xbf": xbf_pack,
            "xf": xf_pack,
        }
        if not b2_zero:
            m["b2g"] = np.tile(np.asarray(b2, BF16).reshape(1, D), (1, LG))
        in_maps.append(m)

    meta = (schedule, woff, nload, nc_tot, b2_zero)
    return in_maps, meta, [ci[4] for ci in core_info]


def _build_program(meta):
    schedule, woff, nload, nc_tot, b2_zero = meta
    f32 = mybir.dt.float32
    bf16 = mybir.dt.bfloat16
    fp8 = mybir.dt.float8e3
    AF = mybir.ActivationFunctionType
    OP = mybir.AluOpType

    nc = bacc.Bacc("TRN2", target_bir_lowering=False, debug=False,
                   num_devices=NCORE)

    NCA = nc_tot + NT + W * SB + P * G
    NCB = D + 6 * P
    ea_d = nc.dram_tensor("ea", [nload, P, L * D], fp8, kind="ExternalInput").ap()
    cbfA_d = nc.dram_tensor("cbfA", [P, NCA], bf16, kind="ExternalInput").ap()
    cbfB_d = nc.dram_tensor("cbfB", [P, NCB], bf16, kind="ExternalInput").ap()
    cf32_d = nc.dram_tensor("cf32", [P, 2], f32, kind="ExternalInput").ap()
    xbf_d = nc.dram_tensor("xbf", [D, NQ * LG * P], fp8, kind="ExternalInput").ap()
    xf_d = nc.dram_tensor("xf", [P, NT * D], bf16, kind="ExternalInput").ap()
    if not b2_zero:
        b2g_d = nc.dram_tensor("b2g", [1, LG * D], bf16, kind="ExternalInput").ap()
    out_d = nc.dram_tensor("outN", [NQ, P, LG * D], bf16, kind="ExternalOutput").ap()

    with tile.TileContext(nc) as tc:
        with (
            tc.tile_pool(name="consts", bufs=1) as consts,
            tc.tile_pool(name="edges", bufs=6) as epool,
            tc.tile_pool(name="xg", bufs=2) as xpool,
            tc.tile_pool(name="yg", bufs=3) as ypool,
            tc.tile_pool(name="s0", bufs=3) as s0pool,
            tc.tile_pool(name="sm", bufs=18) as spool,
            tc.tile_pool(name="work", bufs=3) as wpool,
            tc.tile_pool(name="ln", bufs=3) as lnpool,
            tc.tile_pool(name="ps", bufs=1, space="PSUM") as pspool,
            tc.tile_pool(name="ps2", bufs=3, space="PSUM") as ps2pool,
            tc.tile_pool(name="psagg", bufs=3, space="PSUM") as psagg,
        ):
            # ---- PE warm-up: zero-dep matmul stream to lift the HAM gate
            # (shares the psagg ring; its bank is recycled by scatter tile 2)
            wz = consts.tile([P, P], bf16, tag="wz")
            nc.vector.memset(wz[:], 0.0)
            wups = psagg.tile([P, P], f32, tag="agg")
            for i in range(N_WARMUP):
                nc.tensor.matmul(wups[:], lhsT=wz[:], rhs=wz[:],
                                 start=(i == 0), stop=(i == N_WARMUP - 1))

            # ---- constants (scalar=ACT HWDGE ring) ----
            cA = consts.tile([P, NCA], bf16, tag="cA")
            nc.scalar.dma_start(out=cA[:], in_=cbfA_d[:])
            cB = consts.tile([P, NCB], bf16, tag="cB")
            nc.scalar.dma_start(out=cB[:], in_=cbfB_d[:])
            cf = consts.tile([P, 2], f32, tag="cf")
            nc.scalar.dma_start(out=cf[:], in_=cf32_d[:])
            xbf_sb = consts.tile([D, NQ * LG * P], fp8, tag="xbf")
            nc.scalar.dma_start(out=xbf_sb[:], in_=xbf_d[:])
            xf_sb = consts.tile([P, NT * D], bf16, tag="xf")
            nc.scalar.dma_start(out=xf_sb[:], in_=xf_d[:])

            o = 0
            jr_sb = cA[:, o:o + nc_tot]; o += nc_tot
            jr0_sb = cA[:, o:o + NT]; o += NT
            iotaw_sb = cA[:, o:o + W * SB]; o += W * SB
            iotag_sb = cA[:, o:o + P * G]
            o = 0
            gb_sb = cB[:, o:o + D]; o += D
            w1xa = cB[:, o:o + P]; o += P
            w1xb = cB[:, o:o + P]; o += P
            w1ga = cB[:, o:o + P]; o += P
            w1gb = cB[:, o:o + P]; o += P
            w2a = cB[:, o:o + P]; o += P
            w2b = cB[:, o:o + P]
            b1a = cf[:, 0:1]
            b1b = cf[:, 1:2]

            eps_sb = consts.tile([P, 1], f32, tag="eps")
            nc.vector.memset(eps_sb[:], LN_EPS)
            if not b2_zero:
                ones_row = consts.tile([1, P], bf16, tag="ones_row")
                nc.vector.memset(ones_row[:], 1.0)
                b2g_sb = consts.tile([1, LG * D], bf16, tag="b2g")
                nc.scalar.dma_start(out=b2g_sb[:], in_=b2g_d[:])

            def mid_bcast(a, shape):
                """AP broadcasting a [P, k] slice to [P, shape[1], k]."""
                return bass.AP(
                    tensor=a.tensor, offset=a.offset,
                    ap=[a.ap[0], [0, shape[1]], a.ap[1]],
                )

            load_tiles = {}

            def ensure_load(ld):
                if ld < 0 or ld >= nload or ld in load_tiles:
                    return
                et = epool.tile([P, L * D], fp8, tag="ea", name=f"ea{ld}")
                nc.sync.dma_start(out=et[:], in_=ea_d[ld])
                load_tiles[ld] = et

            def edge_slice(c):
                ld, sl = divmod(c, L)
                ensure_load(ld)
                ensure_load(ld + 1)
                ensure_load(ld + 2)
                return load_tiles[ld][:, sl * D : (sl + 1) * D]

            chunk_base = np.zeros(NT + 1, dtype=np.int64)
            np.cumsum(schedule, out=chunk_base[1:])

            # batched full-width S for the chunk-0s of G tiles,
            # layout [e, n, t] (t innermost -> 2x mode)
            s0_tiles = {}

            def s0_group(gi):
                if gi not in s0_tiles:
                    S0g = s0pool.tile([P, P * G], bf16, tag="S0g")
                    jr0s = jr0_sb[:, gi * G : (gi + 1) * G]
                    nc.vector.tensor_tensor(
                        out=S0g[:].rearrange("p (n t) -> p n t", t=G),
                        in0=mid_bcast(jr0s, [P, P, G]),
                        in1=iotag_sb.rearrange("p (n t) -> p n t", t=G),
                        op=OP.is_equal,
                    )
                    s0_tiles[gi] = S0g
                return s0_tiles[gi]

            def s0_rhs(gi, ti):
                S0g = s0_group(gi)
                a = S0g[:]
                return bass.AP(tensor=a.tensor, offset=a.offset + ti,
                               ap=[a.ap[0], [G, P]])

            aggT_pairs = {}
            s_of = {}

            def sbuild_tile(t):
                """Selection matrices for tile t, layout [e, w, q]."""
                c0 = int(chunk_base[t])
                ncch = int(schedule[t])
                s0_group(t // G)
                sbs = []
                for q0 in range(1, ncch, SB):
                    qn = min(SB, ncch - q0)
                    Sb = spool.tile([P, W * SB], bf16, tag="Sb",
                                    name=f"Sb{t}_{q0}")
                    jrs = jr_sb[:, c0 + q0 : c0 + q0 + qn]
                    nc.vector.tensor_tensor(
                        out=Sb[:, : W * qn].rearrange("p (w q) -> p w q", q=qn),
                        in0=mid_bcast(jrs, [P, W, qn]),
                        in1=bass.AP(tensor=iotaw_sb.tensor,
                                    offset=iotaw_sb.offset,
                                    ap=[iotaw_sb.ap[0], [SB, W], [1, qn]]),
                        op=OP.is_equal,
                    )
                    sbs.append((Sb, qn))
                s_of[t] = sbs

            def win_rhs(Sb, qn, i):
                a = Sb[:]
                return bass.AP(tensor=a.tensor, offset=a.offset + i,
                               ap=[a.ap[0], [qn, W]])

            def scatter_tile(t):
                gi, ti = divmod(t, G)
                c0 = int(chunk_base[t])
                ncch = int(schedule[t])
                agg_ps = psagg.tile([P, P], f32, tag="agg")
                nc.tensor.matmul(
                    agg_ps[:], lhsT=edge_slice(c0), rhs=s0_rhs(gi, ti),
                    start=True, stop=(ncch == 1),
                )
                sbs = s_of.pop(t)
                for bi, q0 in enumerate(range(1, ncch, SB)):
                    Sb, qn = sbs[bi]
                    for i in range(qn):
                        c = c0 + q0 + i
                        w = int(woff[c])
                        nc.tensor.matmul(
                            agg_ps[:, w : w + W],
                            lhsT=edge_slice(c),
                            rhs=win_rhs(Sb, qn, i),
                            start=False,
                            stop=(c == c0 + ncch - 1),
                            skip_group_check=True,
                        )
                # copy to SBUF so the PSUM bank frees early; quads of tiles
                # share one SBUF tile so h1 can batch over all four.
                # GpSimd cannot read PSUM; split copies between ACT and DVE.
                p, half = divmod(t, LG)
                if half == 0:
                    aggT_pairs[p] = wpool.tile([P, LG * P], bf16, tag="aggT",
                                               name=f"aggT{p}")
                dst = aggT_pairs[p][:, half * P : (half + 1) * P]
                if t % 2 == 0:
                    nc.scalar.activation(out=dst, in_=agg_ps[:],
                                         func=AF.Copy, bias=0.0, scale=1.0)
                else:
                    nc.vector.tensor_copy(out=dst, in_=agg_ps[:])

            def mlp_h1_quad(p):
                """h1 for tiles 4p..4p+3 batched over the node axis."""
                t0 = LG * p
                nt = min(LG, NT - t0)
                aggT = aggT_pairs.pop(p)
                NN = nt * P
                xT = xbf_sb[:, p * LG * P : p * LG * P + NN]

                h1a_ps = pspool.tile([P, LG * P], f32, tag="h1a")
                nc.tensor.matmul(h1a_ps[:, 0:NN], lhsT=w1xa, rhs=xT,
                                 start=True, stop=False)
                nc.tensor.matmul(h1a_ps[:, 0:NN], lhsT=w1ga,
                                 rhs=aggT[:, 0:NN], start=False, stop=True)
                h1a = wpool.tile([P, LG * P], bf16, tag="h1a_sb")
                nc.scalar.activation(out=h1a[:, 0:NN], in_=h1a_ps[:, 0:NN],
                                     func=AF.Relu, bias=b1a, scale=1.0)

                h1b_ps = pspool.tile([P, LG * P], f32, tag="h1b")
                nc.tensor.matmul(h1b_ps[:, 0:NN], lhsT=w1xb, rhs=xT,
                                 start=True, stop=False)
                nc.tensor.matmul(h1b_ps[:, 0:NN], lhsT=w1gb,
                                 rhs=aggT[:, 0:NN], start=False, stop=True)
                h1b = wpool.tile([P, LG * P], bf16, tag="h1b_sb")
                nc.scalar.activation(out=h1b[:, 0:NN], in_=h1b_ps[:, 0:NN],
                                     func=AF.Relu, bias=b1b, scale=1.0)
                return h1a, h1b

            # ---- h2 into a 4-tile PSUM group, LN tail per group ----
            ln_state = {}

            def h2_tile(t, h1a, h1b, half):
                lg, li = divmod(t, LG)
                if li == 0:
                    ln_state[lg] = ps2pool.tile([P, LG * P], f32, tag="h2g",
                                                name=f"h2g{lg}")
                h2g = ln_state[lg]
                sl = slice(li * P, (li + 1) * P)
                # start=True clears the has_written bits of the whole PSUM
                # BANK, so only the group's first matmul may set it; later
                # slices rely on the bank-wide clear (first write with
                # start=False overwrites where has_written=0)
                nc.tensor.matmul(h2g[:, sl],
                                 lhsT=h1a[:, half * P : (half + 1) * P],
                                 rhs=w2a, start=(li == 0), stop=False,
                                 skip_group_check=(li != 0))
                last = (li == LG - 1) or (t == NT - 1)
                nc.tensor.matmul(h2g[:, sl],
                                 lhsT=h1b[:, half * P : (half + 1) * P],
                                 rhs=w2b, start=False,
                                 stop=(b2_zero and last),
                                 skip_group_check=True)

            def ln_group(lg):
                """b2 + LayerNorm + gamma + residual + store, tiles
                [4*lg, 4*lg+nt)."""
                t0 = lg * LG
                nt = min(LG, NT - t0)
                h2g = ln_state[lg]
                NN = nt * P
                # rank-1 b2 add over the whole group, closes all accum
                # groups.  Skipped when b2 == 0.
                if not b2_zero:
                    nc.tensor.matmul(h2g[:, 0:NN], lhsT=ones_row[:],
                                     rhs=b2g_sb[:, 0:NN], start=False,
                                     stop=True, skip_group_check=True)
                # LN stats directly on PSUM: one batched bn_stats, per-tile
                # bn_aggr
                stats = lnpool.tile([P, LG * 6], f32, tag="stats")
                mv = lnpool.tile([P, LG * 2], f32, tag="mv")
                for i in range(nt):
                    nc.vector.bn_stats(out=stats[:, 6 * i : 6 * i + 6],
                                       in_=h2g[:, i * P : (i + 1) * P])
                    nc.vector.bn_aggr(out=mv[:, 2 * i : 2 * i + 2],
                                      in_=stats[:, 6 * i : 6 * i + 6])
                mva = mv[:]
                var_sl = bass.AP(tensor=mva.tensor, offset=mva.offset + 1,
                                 ap=[mva.ap[0], [2, nt]])
                mean_sl = bass.AP(tensor=mva.tensor, offset=mva.offset,
                                  ap=[mva.ap[0], [2, nt]])
                rstd = lnpool.tile([P, LG], f32, tag="rstd")
                nc.scalar.activation(out=rstd[:, 0:nt], in_=var_sl,
                                     func=AF.Sqrt, bias=eps_sb[:], scale=1.0)
                nc.vector.reciprocal(out=rstd[:, 0:nt], in_=rstd[:, 0:nt])
                # nmr = -mu * rstd  (bias for the ACT Identity apply)
                nmr = lnpool.tile([P, LG], f32, tag="nmr")
                nc.vector.tensor_tensor(out=nmr[:, 0:nt], in0=mean_sl,
                                        in1=rstd[:, 0:nt], op=OP.mult)
                nc.vector.tensor_scalar(out=nmr[:, 0:nt], in0=nmr[:, 0:nt],
                                        scalar1=-1.0, scalar2=None,
                                        op0=OP.mult)
                # u = v*rstd - mu*rstd per tile on ACT, reading PSUM
                u_g = lnpool.tile([P, LG * D], bf16, tag="u_g")
                for i in range(nt):
                    nc.scalar.activation(
                        out=u_g[:, i * D : (i + 1) * D],
                        in_=h2g[:, i * P : (i + 1) * P],
                        func=AF.Identity,
                        bias=nmr[:, i : i + 1],
                        scale=rstd[:, i : i + 1],
                    )
                del ln_state[lg]
                # gamma mult + residual add on GpSimd, store on sync ring
                t2 = ypool.tile([P, LG * D], bf16, tag="t2")
                gba = gb_sb
                veng = nc.vector if lg == NQ - 1 else nc.gpsimd
                veng.tensor_tensor(
                    out=t2[:, 0:NN].rearrange("p (t f) -> p t f", f=D),
                    in0=u_g[:, 0:NN].rearrange("p (t f) -> p t f", f=D),
                    in1=bass.AP(tensor=gba.tensor, offset=gba.offset,
                                ap=[gba.ap[0], [0, nt], [1, D]]),
                    op=OP.mult,
                )
                y_g = ypool.tile([P, LG * D], bf16, tag="yg")
                veng.tensor_tensor(out=y_g[:, 0:NN], in0=t2[:, 0:NN],
                                   in1=xf_sb[:, t0 * D : t0 * D + NN],
                                   op=OP.add)
                nc.sync.dma_start(out=out_d[lg][:, 0:NN], in_=y_g[:, 0:NN])

            # software pipeline: S-builds run SA tiles ahead of the scatter
            # matmuls, which run MA tiles ahead of the MLP
            SA, MA = 16, 6
            for t in range(min(SA, NT)):
                sbuild_tile(t)
            for t in range(min(MA, NT)):
                scatter_tile(t)
            for p in range(NQ):
                t0 = LG * p
                quad = [t for t in range(t0, t0 + LG) if t < NT]
                for t in quad:
                    if t + SA < NT:
                        sbuild_tile(t + SA)
                h1a, h1b = mlp_h1_quad(p)
                for t in quad:
                    if t + MA < NT:
                        scatter_tile(t + MA)
                for t in quad:
                    h2_tile(t, h1a, h1b, t - t0)
                ln_group(p)

    nc.finalize()
    return nc


LAST_RESULT = None


def kernel(x, edge_index, edge_attr, W1, b1, W2, b2, ln_g, ln_b):
    global LAST_RESULT
    in_maps, meta, tile_perms = _prep_host(
        x, edge_index, edge_attr, W1, b1, W2, b2, ln_g, ln_b
    )
    nc = _build_program(meta)
    trace = bool(os.environ.get("KERNEL_TRACE"))
    res = run_bass_kernel_spmd(
        nc, in_maps, core_ids=list(range(NCORE)), trace=trace
    )
    LAST_RESULT = res

    out = np.empty((N_NODES, D), dtype=np.float32)
    for c in range(NCORE):
        yN = np.asarray(res.results[c]["outN"], dtype=np.float32)
        y_slots = (
            yN.reshape(NQ, P, LG, D).transpose(0, 2, 1, 3)
            .reshape(NQ * LG, P, D)[:NT]
        )
        y_tiles = np.empty_like(y_slots)
        y_tiles[tile_perms[c]] = y_slots
        y = y_tiles.reshape(NPAD, D)[:NSHARD]
        out[c * NSHARD : (c + 1) * NSHARD] = y
    return out
